# revision 1
# baseline (speedup 1.0000x reference)
"""Trainium2 Bass kernel for a single nGPT-style attention head.

Computation (see reference): fused QKV projection, RoPE over the full head
dim, L2-normalize q/k scaled by sqk, causal SDPA with scale sqrt(d_model).

Sharding: data-parallel over batch — 8 batch elements, one per NeuronCore.
Each core gets x[b] (pre-transposed to [C, T] on the host so the
contraction dim lands on SBUF partitions, cast to bf16 — all on-chip
compute is bf16 with fp32 PSUM accumulation anyway), the shared QKV weight
(pre-transposed to [C, 3D], bf16), RoPE cos/sin tables, a causal triangle
mask tile and sqk. The core computes out^T = [D, T]; the host transposes
back and stacks.

Structure (three phases; phases pipeline internally per 512-token block):
  A: QKV matmuls (bf16) -> psum; copies to bf16 q^T|k^T|v^T; row norms via
     bf16 square (GPSIMD) + ones-matmul partition reduction; 1/||.|| via
     ACT Ln + Exp(-0.5 ln) — same ACT table set as the attention exp, so
     the activation tables load exactly once.
  B: RoPE on the concatenated q|k [128, 2T] (DVE + GPSIMD split), inverse
     norm broadcast across partitions with GPSIMD partition_broadcast,
     sqrt(C)*sqk^2 folded into q; v transposed [d,t]->[t,d] per 128-tile
     via DRAM-roundtrip XBAR transpose DMA.
  C: per tq-block: scores^T strips (k-tile stationary) -> PSUM, ACT exp ->
     bf16, causal triangle handled by a [128,128] mask multiply + zeroed
     prefix, attn@v (v-tile stationary) and ones-denominator matmuls
     accumulate out^T and row sums; softmax division via ACT Ln/Exp
     reciprocal + GPSIMD broadcast + one DVE multiply.
"""

import numpy as np
import ml_dtypes

import concourse.bass as bass
import concourse.tile as tile
from concourse import bacc, mybir
from concourse.bass import ts, ds
from concourse.bass_utils import run_bass_kernel_spmd

# Surface compile-hook exceptions (the PJRT bridge swallows tracebacks).
try:
    import traceback
    import libneuronxla as _lnx

    if not getattr(_lnx, "_err_wrapped", False):
        _orig_cc = _lnx.neuronx_cc

        def _cc_wrapper(*a, **kw):
            try:
                return _orig_cc(*a, **kw)
            except BaseException:
                traceback.print_exc()
                raise

        _lnx.neuronx_cc = _cc_wrapper
        _lnx._err_wrapped = True
except Exception:
    pass

AFT = mybir.ActivationFunctionType
ALU = mybir.AluOpType
F32 = mybir.dt.float32
BF16 = mybir.dt.bfloat16

B, T_FULL, C, D = 8, 2048, 1024, 128
ROPE_BASE = 10000.0
P = 128
TB = 512  # t-block (tq block width, PSUM-bank free dim)
NCO = C // P  # contraction chunks for the QKV projection


def build_nc(T=T_FULL, num_devices=8):
    from contextlib import ExitStack
    NTB = T // TB
    NKT = T // P
    nc = bacc.Bacc("TRN2", target_bir_lowering=False, debug=False,
                   num_devices=num_devices)

    xT = nc.dram_tensor("xT", [C, T], BF16, kind="ExternalInput").ap()
    WT = nc.dram_tensor("WT", [C, 3 * D], BF16, kind="ExternalInput").ap()
    cosF = nc.dram_tensor("cosF", [P, 2 * T], BF16, kind="ExternalInput").ap()
    sinF = nc.dram_tensor("sinF", [P, 2 * T], BF16, kind="ExternalInput").ap()
    tri = nc.dram_tensor("tri", [P, P], BF16, kind="ExternalInput").ap()
    sqk = nc.dram_tensor("sqk", [D, 1], F32, kind="ExternalInput").ap()
    zro = nc.dram_tensor("zro", [P, 3 * P], BF16, kind="ExternalInput").ap()
    onb = nc.dram_tensor("onb", [P, 1], BF16, kind="ExternalInput").ap()
    outT = nc.dram_tensor("outT", [D, T], F32, kind="ExternalOutput").ap()

    xT_t = xT.rearrange("(co p) t -> p co t", p=P)
    WT_t = WT.rearrange("(co p) d -> p co d", p=P)
    H = P // 2

    with tile.TileContext(nc) as tc:
        with ExitStack() as ctx:
            const = ctx.enter_context(tc.tile_pool(name="const", bufs=1))
            wpool = ctx.enter_context(tc.tile_pool(name="wpool", bufs=2))
            dramp = ctx.enter_context(
                tc.tile_pool(name="dramp", bufs=1, space="DRAM"))

            wt = const.tile([P, NCO, 3 * D], BF16)
            nc.sync.dma_start(wt, WT_t)
            sqk_sb = const.tile([D, 1], F32)
            nc.sync.dma_start(sqk_sb, sqk)
            ones_k = const.tile([P, 1], BF16)
            nc.sync.dma_start(ones_k, onb)
            tri_sb = const.tile([P, P], BF16)
            nc.sync.dma_start(tri_sb, tri)
            # (sqk * C^(1/4))^2 = sqrt(C) * sqk^2 — full logit scale, on q.
            sqk232 = const.tile([D, 1], F32)
            nc.vector.tensor_scalar_mul(sqk232, sqk_sb, float(C ** 0.25))
            nc.vector.tensor_mul(sqk232, sqk232, sqk232)

            qk = const.tile([P, 2 * T], BF16)   # q̃^T | k̃^T
            vst = const.tile([P, T], BF16)      # v^T staging
            vt = const.tile([P, NKT, P], BF16)  # v tiles [tk, e]
            invn = const.tile([1, 2 * T], F32)  # 1/||q||, 1/||k||
            vd = dramp.tile([P, T], BF16)

            # ---------- Phase A: QKV + norms + RoPE (per block) ----------
            cos_sb = const.tile([P, 2 * T], BF16)
            sin_sb = const.tile([P, 2 * T], BF16)
            with ExitStack() as actx:
                xpool = actx.enter_context(
                    tc.tile_pool(name="xpool", bufs=18))
                ps_qkv = actx.enter_context(
                    tc.tile_pool(name="ps_qkv", bufs=3, space="PSUM"))
                ps_n = actx.enter_context(
                    tc.tile_pool(name="ps_n", bufs=2, space="PSUM"))
                for j in range(NTB):
                    with nc.named_scope(f"qkv{j}"):
                        xts = []
                        for co in range(NCO):
                            xt = xpool.tile([P, TB], BF16, tag="xt")
                            nc.sync.dma_start(xt, xT_t[:, co, ts(j, TB)])
                            xts.append(xt)
                        for g in range(3):
                            ps = ps_qkv.tile([P, TB], F32, tag="qkv")
                            for co in range(NCO):
                                nc.tensor.matmul(
                                    ps, wt[:, co, ts(g, D)], xts[co],
                                    start=(co == 0), stop=(co == NCO - 1))
                            if g < 2:
                                dst = qk[:, ds(g * T + j * TB, TB)]
                                nc.scalar.activation(dst, ps, AFT.Copy)
                                sq = wpool.tile([P, TB], BF16, tag="sq")
                                nc.gpsimd.tensor_mul(sq, dst, dst)
                                nps = ps_n.tile([1, TB], F32, tag="n")
                                nc.tensor.matmul(nps, ones_k, sq,
                                                 start=True, stop=True)
                                lnn = wpool.tile([1, TB], F32, tag="lnn")
                                nc.scalar.activation(lnn, nps, AFT.Ln)
                                nc.scalar.activation(
                                    invn[:, ds(g * T + j * TB, TB)], lnn,
                                    AFT.Exp, scale=-0.5)
                            else:
                                nc.scalar.activation(
                                    vst[:, ds(j * TB, TB)], ps, AFT.Copy)

                    with nc.named_scope(f"rope{j}"):
                        for part in range(2):  # 0 = q chunk, 1 = k chunk
                            ofs = part * T + j * TB
                            ch = ds(ofs, TB)
                            nc.sync.dma_start(cos_sb[:, ch], cosF[:, ch])
                            nc.sync.dma_start(sin_sb[:, ch], sinF[:, ch])
                            bcst = wpool.tile([P, TB], F32, tag="bcst")
                            nc.gpsimd.partition_broadcast(bcst, invn[:, ch])
                            # rotate_half: the partition move runs on the
                            # otherwise-idle GPSIMD (1-input ops are cheap
                            # there; 2-input SBUF ops require equal base
                            # partitions, so this can't fold into the mul).
                            rot = wpool.tile([P, TB], BF16, tag="rot")
                            nc.gpsimd.tensor_copy(rot[0:H, :], qk[H:P, ch])
                            nc.gpsimd.tensor_copy(rot[H:P, :], qk[0:H, ch])
                            t2 = wpool.tile([P, TB], BF16, tag="t2")
                            nc.vector.tensor_mul(t2, rot, sin_sb[:, ch])
                            t1 = wpool.tile([P, TB], BF16, tag="t1")
                            nc.vector.tensor_mul(t1, qk[:, ch], cos_sb[:, ch])
                            nc.vector.tensor_add(t1, t1, t2)
                            if part == 0:
                                nc.vector.scalar_tensor_tensor(
                                    out=qk[:, ch], in0=t1, scalar=sqk232,
                                    in1=bcst, op0=ALU.mult, op1=ALU.mult)
                            else:
                                nc.vector.tensor_mul(qk[:, ch], t1, bcst)

                        # v transpose via DRAM-roundtrip XBAR DMA (bf16)
                        nc.sync.dma_start(vd[:, ts(j, TB)],
                                          vst[:, ts(j, TB)])
                        for i in range(4 * j, 4 * j + 4):
                            nc.sync.dma_start_transpose(vt[:, i, :],
                                                        vd[:, ts(i, P)])

            # ---------- Phase C: causal attention ----------
            with ExitStack() as cctx:
                expool = cctx.enter_context(
                    tc.tile_pool(name="expool", bufs=3))
                ps_sc = cctx.enter_context(
                    tc.tile_pool(name="ps_sc", bufs=2, space="PSUM"))
                ps_o = cctx.enter_context(
                    tc.tile_pool(name="ps_o", bufs=2, space="PSUM"))
                ps_d = cctx.enter_context(
                    tc.tile_pool(name="ps_d", bufs=2, space="PSUM"))

                for J in range(NTB):
                    with nc.named_scope(f"att{J}"):
                        q_blk = qk[:, ts(J, TB)]
                        po = ps_o.tile([P, TB], F32, tag="o")
                        pd = ps_d.tile([1, TB], F32, tag="d")
                        nstr = (TB // P) * (J + 1)
                        exs = {}

                        def emit_scores(g, J=J, q_blk=q_blk, exs=exs):
                            strips = [2 * g, 2 * g + 1]
                            sc = ps_sc.tile([P, 2, TB], F32, tag="sc")
                            ex = expool.tile([P, 2, TB], BF16, tag="ex")
                            offs = []
                            for r2, i in enumerate(strips):
                                dr = i - (TB // P) * J
                                off = P * dr if dr >= 0 else 0
                                offs.append(off)
                                nc.tensor.matmul(
                                    sc[:, r2, ds(off, TB - off)],
                                    qk[:, ds(T + P * i, P)],
                                    q_blk[:, ds(off, TB - off)],
                                    start=True, stop=True)
                            if offs == [0, 0]:
                                nc.scalar.activation(ex, sc, AFT.Exp)
                            else:
                                for r2, i in enumerate(strips):
                                    off = offs[r2]
                                    nc.scalar.activation(
                                        ex[:, r2, ds(off, TB - off)],
                                        sc[:, r2, ds(off, TB - off)],
                                        AFT.Exp)
                            for r2, i in enumerate(strips):
                                off = offs[r2]
                                if i - (TB // P) * J >= 0:
                                    nc.vector.tensor_mul(
                                        ex[:, r2, ds(off, P)],
                                        ex[:, r2, ds(off, P)], tri_sb)
                                    if off > 0:
                                        nc.sync.dma_start(
                                            ex[:, r2, ds(0, off)],
                                            zro[:, ds(0, off)])
                            exs[g] = ex

                        def emit_av(g, nstr=nstr, po=po, pd=pd, exs=exs):
                            ex = exs.pop(g)
                            for r2, i in enumerate((2 * g, 2 * g + 1)):
                                nc.tensor.matmul(
                                    po, vt[:, i, :], ex[:, r2, :],
                                    start=(i == 0), stop=(i == nstr - 1))
                                nc.tensor.matmul(
                                    pd, ones_k, ex[:, r2, :],
                                    start=(i == 0), stop=(i == nstr - 1))

                        ngr = nstr // 2
                        emit_scores(0)
                        for g in range(1, ngr):
                            emit_scores(g)
                            emit_av(g - 1)
                        emit_av(ngr - 1)

                        lnd = wpool.tile([1, TB], F32, tag="lnd")
                        nc.scalar.activation(lnd, pd, AFT.Ln)
                        invd = wpool.tile([1, TB], F32, tag="invd")
                        nc.scalar.activation(invd, lnd, AFT.Exp, scale=-1.0)
                        bc2s = wpool.tile([P, TB], F32, tag="bc2s")
                        nc.gpsimd.partition_broadcast(bc2s, invd)
                        ob = wpool.tile([P, TB], F32, tag="ob")
                        nc.vector.tensor_mul(ob, po, bc2s)
                        nc.sync.dma_start(outT[:, ts(J, TB)], ob)

    nc.compile()
    return nc


def _host_tables(T):
    d = D
    inv_freq = 1.0 / (ROPE_BASE ** (np.arange(0, d, 2, dtype=np.float64) / d))
    t = np.arange(T, dtype=np.float64)
    freqs = np.outer(inv_freq, t)  # [d/2, T]
    emb = np.concatenate([freqs, freqs], axis=0)  # [d, T]
    cos1 = np.cos(emb)
    sin1 = np.sin(emb)
    # sign of rotate_half folded into the table: rot is built with plain
    # copies, and sin rows 0:d/2 carry the minus sign instead.
    sin1[: d // 2, :] *= -1.0
    cosF = np.concatenate([cos1, cos1], axis=1).astype(ml_dtypes.bfloat16)
    sinF = np.concatenate([sin1, sin1], axis=1).astype(ml_dtypes.bfloat16)
    a = np.arange(P)
    tri = (a[None, :] >= a[:, None]).astype(ml_dtypes.bfloat16)  # [tk, tq]
    return cosF, sinF, tri


TRACE = False
LAST_EXEC_NS = None
LAST_TRACE = None
LAST_INSTS = None


def kernel(x, W_qkv, sqk):
    global LAST_EXEC_NS, LAST_TRACE, LAST_INSTS
    T = x.shape[1]
    cosF, sinF, tri = _host_tables(T)
    WT = np.ascontiguousarray(np.asarray(W_qkv).T).astype(ml_dtypes.bfloat16)
    sqk2 = np.ascontiguousarray(
        np.asarray(sqk).reshape(D, 1)).astype(np.float32)
    in_maps = []
    for b in range(B):
        in_maps.append({
            "xT": np.ascontiguousarray(
                np.asarray(x[b]).T).astype(ml_dtypes.bfloat16),
            "WT": WT,
            "cosF": cosF,
            "sinF": sinF,
            "tri": tri,
            "sqk": sqk2,
            "zro": np.zeros((P, 3 * P), ml_dtypes.bfloat16),
            "onb": np.ones((P, 1), ml_dtypes.bfloat16),
        })
    nc = build_nc(T=T, num_devices=B)
    res = run_bass_kernel_spmd(nc, in_maps, core_ids=list(range(B)),
                               trace=TRACE)
    LAST_EXEC_NS = res.exec_time_ns
    LAST_TRACE = (res.instructions_and_trace[1]
                  if res.instructions_and_trace else None)
    LAST_INSTS = (res.instructions_and_trace[0]
                  if res.instructions_and_trace else None)
    out = np.stack([r["outT"].T for r in res.results])  # [B, T, D]
    return np.ascontiguousarray(out).astype(np.float32)

